# revision 15
# baseline (speedup 1.0000x reference)
"""Self-contained TRN2 Bass kernel for the COR Critic network.

kernel(**inputs) takes the FULL (unsharded) numpy inputs keyed as in
setup_inputs() and returns the FULL [131072, 1] float32 output.

Sharding: pure data parallel over 8 NeuronCores - the batch dim of
state/action is split into 8 equal shards; the (tiny) weights are
replicated. No collectives are needed; per-core outputs are
concatenated on the host.

Implementation notes (per 512-row super-tile, per core):
  - the whole network runs fused on-chip; no intermediate HBM traffic
  - matmul operands in fp16 (PSUM accumulation is fp32); LayerNorm
    statistics and normalization are computed in fp32
  - LayerNorm rstd via DVE Newton iterations (bit-trick seed), keeping
    the ACT engine inside a single activation-table set (tanh/relu)
  - sigmoid gates are folded into the next layer's weight rows on the
    host (pure input marshalling), so no on-chip preamble math
  - LN1 transposes ride the DMA XBAR (dma_start_transpose), not the PE;
    the freed PSUM banks deepen the r2 accumulator rotation (psA=5)
  - three-stage software pipeline: A(p) [r1 riders + r2 + q1 + LN1
    stats] -> TR(p) [XBAR transpose + ACT relu] -> Q2/Btail(p-1); the
    q2 matmuls of pair p-1 issue after pair p's heavy matmuls so the
    PE never waits on the LN1 chain
  - final [128,128] output stays untransposed on-chip; the host
    transposes during unmarshalling
"""

import os

os.environ.setdefault("BASS_NEVER_TRACE", "1")

import numpy as np

import concourse.bacc as bacc
import concourse.bass as bass
import concourse.tile as tile
from concourse import mybir
from concourse.masks import make_identity

F32 = mybir.dt.float32
F32R = mybir.dt.float32r
F16 = mybir.dt.float16
I32 = mybir.dt.int32

# matmul-operand dtype: fp16 halves weight-load time (and enables FWL)
# at ~2e-4 relative rounding; all LayerNorm math stays fp32.
USE_FP16 = True
MMDT = F16 if USE_FP16 else F32R
MMNP = "float16" if USE_FP16 else "float32"
RSQRT_MAGIC = 0x5F3759DF

N_CORES = 8
B_CORE = 16384  # batch rows per core
T = 512         # super-tile batch rows
N_TILES = B_CORE // T
EPS = 1e-5


def build_nc():
    nc = bacc.Bacc("TRN2", target_bir_lowering=False, debug=False,
                   num_devices=N_CORES)

    # DRAM I/O (shapes match host-side pre-marshalled arrays)
    sa = nc.dram_tensor("sa", [N_TILES // 2, 64, T], MMDT, kind="ExternalInput").ap()
    w1 = nc.dram_tensor("w1", [64, 1024], MMDT, kind="ExternalInput").ap()
    b1 = nc.dram_tensor("b1", [128, 8], F32, kind="ExternalInput").ap()
    # w2 pre-chunked on host along the j (output-feature) axis so the
    # first chunk unblocks ripple-2 j=0 early
    w2c = [nc.dram_tensor(f"w2c{c}", [128, 8, 256], MMDT,
                          kind="ExternalInput").ap() for c in range(4)]
    b2 = nc.dram_tensor("b2", [128, 8], F32, kind="ExternalInput").ap()
    wq1 = nc.dram_tensor("wq1", [128, 8, 256], MMDT, kind="ExternalInput").ap()
    bq1 = nc.dram_tensor("bq1", [128, 256], F32, kind="ExternalInput").ap()
    l1g = nc.dram_tensor("l1g", [128, 2], F32, kind="ExternalInput").ap()
    l1b = nc.dram_tensor("l1b", [128, 2], F32, kind="ExternalInput").ap()
    wq2 = nc.dram_tensor("wq2", [128, 2, 128], MMDT, kind="ExternalInput").ap()
    bq2 = nc.dram_tensor("bq2", [128, 128], F32, kind="ExternalInput").ap()
    l2g = nc.dram_tensor("l2g", [128, 4, 128], F32, kind="ExternalInput").ap()
    l2b = nc.dram_tensor("l2b", [128, 4, 128], F32, kind="ExternalInput").ap()
    wq3 = nc.dram_tensor("wq3", [128, 4, 128], F32, kind="ExternalInput").ap()
    bq3 = nc.dram_tensor("bq3", [128, 1], F32, kind="ExternalInput").ap()
    y = nc.dram_tensor("y", [128, 128], F32, kind="ExternalOutput").ap()

    AF = mybir.ActivationFunctionType
    OP = mybir.AluOpType

    with tile.TileContext(nc) as tc:
        with (
            tc.tile_pool(name="consts", bufs=1) as consts,
            tc.tile_pool(name="acts", bufs=2) as acts,
            tc.tile_pool(name="work", bufs=3) as work,
            tc.tile_pool(name="psA", bufs=5, space="PSUM") as psA,
            tc.tile_pool(name="psB", bufs=3, space="PSUM") as psB,
        ):
            # ---------------- preamble: weights to SBUF ----------------
            # All on the sync HWDGE queue; issue order IS the priority
            # order (first slab + r1 weights first so the PE starts
            # within ~3us, then w2 chunk 0 which gates ripple-2 j=0).
            def load(name, shape, dt, src):
                t_ = consts.tile(shape, dt, tag=name)
                nc.sync.dma_start(out=t_, in_=src)
                return t_

            sa2_0 = work.tile([64, T], MMDT, tag="sa_fm")
            nc.sync.dma_start(out=sa2_0, in_=sa[0])
            w1_sb = load("w1", [64, 1024], MMDT, w1)
            b1_sb = load("b1", [128, 8], F32, b1)
            b2_sb = load("b2", [128, 8], F32, b2)
            w2_sb = consts.tile([128, 8, 1024], MMDT, tag="w2")
            for c in range(4):
                nc.sync.dma_start(out=w2_sb[:, :, c * 256:(c + 1) * 256],
                                  in_=w2c[c])
            wq1_sb = load("wq1", [128, 8, 256], MMDT, wq1)
            bq1_sb = load("bq1", [128, 256], F32, bq1)
            l1g_sb = load("l1g", [128, 2], F32, l1g)
            l1b_sb = load("l1b", [128, 2], F32, l1b)
            wq2_sb = load("wq2", [128, 2, 128], MMDT, wq2)
            bq2_sb = load("bq2", [128, 128], F32, bq2)
            l2g_sb = load("l2g", [128, 4, 128], F32, l2g)
            l2b_sb = load("l2b", [128, 4, 128], F32, l2b)
            wq3_sb = load("wq3", [128, 4, 128], F32, wq3)
            bq3_sb = load("bq3", [128, 1], F32, bq3)

            y_all = consts.tile([128, 128], F32, tag="y_all")
            nc.vector.memset(y_all, 0.0)
            magic = consts.tile([128, 4], I32)
            nc.vector.memset(magic, RSQRT_MAGIC)
            # fp16 identity for the last-pair PE-transpose fast path
            ident = consts.tile([128, 128], F32)
            make_identity(nc, ident)
            ident16 = consts.tile([128, 128], MMDT)
            nc.vector.tensor_copy(ident16, ident)

            # Newton rsqrt on DVE (avoids ACT Sqrt: bad ULP + a table-set
            # swap against Tanh every tile). vars_ap: [128, n] variances.
            def rsqrt_dve(vars_ap, n):
                v = work.tile([128, 4], F32, tag="rsq_v")
                nc.vector.tensor_scalar_add(v[:, :n], in0=vars_ap, scalar1=EPS)
                ti = work.tile([128, 4], I32, tag="rsq_t")
                nc.vector.tensor_scalar(
                    ti[:, :n], in0=v[:, :n].bitcast(I32), scalar1=1,
                    scalar2=None, op0=OP.logical_shift_right)
                yn = work.tile([128, 4], F32, tag="rsq_y")
                nc.vector.tensor_sub(yn[:, :n].bitcast(I32), in0=magic[:, :n],
                                     in1=ti[:, :n])
                for _ in range(3):
                    a = work.tile([128, 4], F32, tag="rsq_a")
                    nc.vector.tensor_mul(a[:, :n], in0=yn[:, :n], in1=yn[:, :n])
                    nc.vector.scalar_tensor_tensor(
                        a[:, :n], in0=a[:, :n], scalar=-0.5, in1=v[:, :n],
                        op0=OP.mult, op1=OP.mult)
                    nc.vector.scalar_tensor_tensor(
                        yn[:, :n], in0=a[:, :n], scalar=1.5, in1=yn[:, :n],
                        op0=OP.add, op1=OP.mult)
                return yn

            # ------------- stage A: matmul-heavy front half -------------
            # Pair-structured. r1 matmuls (K=32, single-shot PSUM whose
            # slot frees only at tanh pace) are interleaved one-per-r2-
            # j-group so their PSUM slot is always free when they issue:
            # tile b's r1 rides tile a's r2; the NEXT pair's tile-a r1
            # rides tile b's r2.
            def r1_chunk(x1, sa2, m, j):
                ps = psA.tile([128, T], F32, tag="mm512")
                nc.tensor.matmul(
                    ps, w1_sb[32 * m:32 * (m + 1), j * 128:(j + 1) * 128],
                    sa2[32 * m:32 * (m + 1), :], start=True, stop=True,
                    tile_position=(32 * m, 0))
                nc.scalar.activation(x1[:, j, :], ps, AF.Tanh,
                                     bias=b1_sb[:, j:j + 1])

            def r2_q1(x1, riders):
                # ripple 2: x2 = tanh(W2f'.T @ x1 + b2)  [1024f, Tb]
                x2 = acts.tile([128, 8, T], MMDT, tag="x2")
                for j in range(8):
                    ps = psA.tile([128, T], F32, tag="mm512")
                    for k in range(8):
                        nc.tensor.matmul(
                            ps, w2_sb[:, k, j * 128:(j + 1) * 128],
                            x1[:, k, :], start=(k == 0), stop=(k == 7))
                    nc.scalar.activation(x2[:, j, :], ps, AF.Tanh,
                                         bias=b2_sb[:, j:j + 1])
                    for r in riders:
                        r1_chunk(*r, j)

                # q1 batch-major: z1 = x2.T @ Wq1' + bq1, then LN1 + norm
                z1sb = work.tile([128, 4, 256], F32, tag="z1sb", bufs=4)
                mv1 = work.tile([128, 4, 2], F32, tag="mv1", bufs=2)
                for cp in range(2):
                    zps2 = psB.tile([128, 2, 256], F32, tag="q1")
                    for ci in range(2):
                        c = 2 * cp + ci
                        for k in range(8):
                            nc.tensor.matmul(
                                zps2[:, ci, :], x2[:, k, c * 128:(c + 1) * 128],
                                wq1_sb[:, k, :], start=(k == 0), stop=(k == 7))
                        nc.vector.tensor_add(z1sb[:, c, :], in0=zps2[:, ci, :],
                                             in1=bq1_sb)
                        st = work.tile([128, 6], F32, tag="st1")
                        nc.vector.bn_stats(st, z1sb[:, c, :])
                        nc.vector.bn_aggr(mv1[:, c, :], st)
                rstd1 = rsqrt_dve(mv1[:, :, 1], 4)
                xn1 = work.tile([128, 4, 256], MMDT, tag="xn1", bufs=4)
                for c in range(4):
                    nc.vector.tensor_scalar(
                        xn1[:, c, :], in0=z1sb[:, c, :],
                        scalar1=mv1[:, c, 0:1], scalar2=rstd1[:, c:c + 1],
                        op0=OP.subtract, op1=OP.mult)
                return xn1

            def stage_A_pair(p, x1_a, sa2):
                # resources for the NEXT pair (its tile-a r1 rides r2_b).
                # sa slabs ride the gpsimd SWDGE queue: the sync queue
                # carries the 16 XBAR transposes per pair, which wait on
                # the LN1 normalize - a slab queued behind them would
                # arrive after the riders need it.
                nxt = None
                if p + 1 < N_TILES // 2:
                    sa2n = work.tile([64, T], MMDT, tag="sa_fm")
                    nc.gpsimd.dma_start(out=sa2n, in_=sa[p + 1])
                    x1an = acts.tile([128, 8, T], MMDT, tag="x1", bufs=3)
                    nxt = (x1an, sa2n)

                x1_b = acts.tile([128, 8, T], MMDT, tag="x1", bufs=3)
                riders = [(x1_b, sa2, 1)]
                if nxt:
                    riders.append((nxt[0], nxt[1], 0))
                xn_a = r2_q1(x1_a, riders)
                xn_b = r2_q1(x1_b, [])
                return nxt, [xn_a, xn_b]

            # ------------- stage TR: XBAR transpose, then LN1 relu -------------
            # xn1 [128b, 4c, 256f] -> xn1T [128f, 2jf, 512b] via 8 DMA
            # XBAR transposes (off the PE, sync HWDGE queue). The
            # per-feature LN1 scale/bias + relu runs on the DVE one
            # pipeline step LATER (stage_relu), by which time the XBAR
            # DMAs have long completed - putting it right here (on any
            # engine) would block that engine's in-order queue on the
            # DMA wait and stall its other work.
            def stage_TRdma(t, xn1):
                xn1T = work.tile([128, 2, T], MMDT, tag="xn1T", bufs=4)
                for c in range(4):
                    for jf in range(2):
                        nc.sync.dma_start_transpose(
                            xn1T[:, jf, c * 128:(c + 1) * 128],
                            xn1[:, c, jf * 128:(jf + 1) * 128])
                return xn1T

            def stage_relu(t, xn1T):
                h1T = work.tile([128, 2, T], MMDT, tag="h1T", bufs=4)
                for jf in range(2):
                    nc.vector.tensor_scalar(
                        h1T[:, jf, :], in0=xn1T[:, jf, :],
                        scalar1=l1g_sb[:, jf:jf + 1],
                        scalar2=l1b_sb[:, jf:jf + 1],
                        op0=OP.mult, op1=OP.add)
                    nc.vector.tensor_scalar_max(
                        h1T[:, jf, :], in0=h1T[:, jf, :], scalar1=0.0)
                return h1T

            # ------------- stage Q2: q2 matmuls + LN2 normalize -------------
            def stage_Q2(t, h1T):
                z2T = work.tile([128, 4, 128], F32, tag="z2T", bufs=4)
                mv2 = work.tile([128, 4, 2], F32, tag="mv2", bufs=2)
                for cp in range(2):
                    zps2 = psB.tile([128, 2, 128], F32, tag="q1")
                    for ci in range(2):
                        c = 2 * cp + ci
                        for k in range(2):
                            nc.tensor.matmul(
                                zps2[:, ci, :], h1T[:, k, c * 128:(c + 1) * 128],
                                wq2_sb[:, k, :], start=(k == 0), stop=(k == 1))
                        nc.vector.tensor_add(z2T[:, c, :], in0=zps2[:, ci, :],
                                             in1=bq2_sb)
                        st2 = work.tile([128, 6], F32, tag="st2")
                        nc.vector.bn_stats(st2, z2T[:, c, :])
                        nc.vector.bn_aggr(mv2[:, c, :], st2)
                rstd2 = rsqrt_dve(mv2[:, :, 1], 4)
                xn2 = work.tile([128, 4, 128], F32, tag="xn2", bufs=4)
                for c in range(4):
                    nc.vector.tensor_scalar(
                        xn2[:, c, :], in0=z2T[:, c, :], scalar1=mv2[:, c, 0:1],
                        scalar2=rstd2[:, c:c + 1],
                        op0=OP.subtract, op1=OP.mult)
                return xn2

            # ------------- stage B tail: q3 on DVE -------------
            # h2 = relu(xn2 * ln2_g + ln2_b); y = h2 . wq3 (+ bq3 added
            # once at the end). Batch-major on the DVE; the elementwise
            # ops run over all 4 c-quarters at once against host-tiled
            # [128, 4, 128] weight replicas. Results collect into
            # y_all[:, t*4+c].
            def stage_Btail(t, xn2):
                h = work.tile([128, 4, 128], F32, tag="hb")
                nc.vector.tensor_mul(h, in0=xn2, in1=l2g_sb)
                nc.vector.tensor_add(h, in0=h, in1=l2b_sb)
                nc.vector.tensor_scalar_max(h, in0=h, scalar1=0.0)
                nc.vector.tensor_mul(h, in0=h, in1=wq3_sb)
                for c in range(4):
                    idx = t * 4 + c
                    nc.vector.reduce_sum(y_all[:, idx:idx + 1], h[:, c, :],
                                         axis=mybir.AxisListType.X)

            # last-pair fast path: PE transpose + ACT relu (the PE is
            # idle in the epilogue and this skips the ~20us XBAR-queue
            # tail latency)
            def stage_TR_pe(t, xn1):
                h1T = work.tile([128, 2, T], MMDT, tag="h1T", bufs=4)
                for c in range(4):
                    for jf in range(2):
                        tp = psA.tile([128, 128], MMDT, tag="mm512")
                        nc.tensor.transpose(
                            tp, xn1[:, c, jf * 128:(jf + 1) * 128], ident16)
                        nc.scalar.activation(
                            h1T[:, jf, c * 128:(c + 1) * 128], tp, AF.Relu,
                            bias=l1b_sb[:, jf:jf + 1],
                            scale=l1g_sb[:, jf:jf + 1])
                return h1T

            # ---------------- software-pipelined batch loop ----------------
            # Issue order per iteration: Q2/Btail(p-2) first (its psB
            # slots must free early for this iteration's q1), then the
            # heavy A(p), then TRdma(p) (XBAR queue), then relu(p-1) -
            # every consumer runs a full pair-iteration after its
            # producer, so no engine queue blocks on another engine's
            # latency.
            NP = N_TILES // 2
            # prologue: pair 0's tile-a r1 runs standalone
            x1a_0 = acts.tile([128, 8, T], MMDT, tag="x1", bufs=3)
            for j in range(8):
                r1_chunk(x1a_0, sa2_0, 0, j)
            pend_a = (x1a_0, sa2_0)
            xtq = {}
            h1q = {}

            def do_relu(p):
                h1q[p] = (stage_relu(2 * p, xtq[p][0]),
                          stage_relu(2 * p + 1, xtq[p][1]))
                del xtq[p]

            def do_q2bt(p):
                for s in range(2):
                    t_ = 2 * p + s
                    xn2 = stage_Q2(t_, h1q[p][s])
                    stage_Btail(t_, xn2)
                del h1q[p]

            for p in range(NP):
                if p >= 2:
                    do_q2bt(p - 2)
                pend_a, xn1_pair = stage_A_pair(p, *pend_a)
                if p < NP - 1:
                    xtq[p] = (stage_TRdma(2 * p, xn1_pair[0]),
                              stage_TRdma(2 * p + 1, xn1_pair[1]))
                else:
                    h1q[p] = (stage_TR_pe(2 * p, xn1_pair[0]),
                              stage_TR_pe(2 * p + 1, xn1_pair[1]))
                if p >= 1 and p - 1 < NP - 1:
                    do_relu(p - 1)
            do_q2bt(NP - 2)
            do_q2bt(NP - 1)
            nc.vector.tensor_scalar_add(y_all, in0=y_all, scalar1=bq3_sb)
            nc.sync.dma_start(out=y, in_=y_all)

    nc.compile()
    return nc


def marshal_inputs(state, action, W1, b1, g1, W2, b2, g2,
                   Wq1, bq1, ln1_g, ln1_b, Wq2, bq2, ln2_g, ln2_b, Wq3, bq3):
    """Host-side layout marshalling (reshape/transpose/tile + gate folds).

    Returns (shared weight map, per-core list of sa slabs)."""
    f32 = np.float32
    B = state.shape[0]
    assert B == N_CORES * B_CORE

    sa = np.concatenate([np.asarray(state, f32), np.asarray(action, f32)],
                        axis=1)  # [B, 32]
    # per-core: [N_TILES//2, 64, T] feature-major pair slabs
    sa_cores = []
    for cid in range(N_CORES):
        s = sa[cid * B_CORE:(cid + 1) * B_CORE]
        sa_cores.append(np.ascontiguousarray(
            s.reshape(N_TILES // 2, 2, T, 32).transpose(0, 1, 3, 2)
            .reshape(N_TILES // 2, 64, T)))

    # sigmoid gates folded into the NEXT layer's weight rows:
    # (tanh(z)*s) @ W == tanh(z) @ (diag(s) W)
    sg1 = 1.0 / (1.0 + np.exp(-np.asarray(g1, np.float64)))
    sg2 = 1.0 / (1.0 + np.exp(-np.asarray(g2, np.float64)))
    sg1r = np.repeat(sg1.astype(f32), 32)  # [1024] per ripple-1 feature
    sg2r = np.repeat(sg2.astype(f32), 32)

    # W1 [H=32, D=32, K=32] -> W1f [D=32, H*K=1024]
    w1f = np.asarray(W1, f32).transpose(1, 0, 2).reshape(32, 1024)
    w1f = np.ascontiguousarray(np.concatenate([w1f, w1f], axis=0))
    # W2 [H=32, D=1024, K=32] -> diag(sg1) @ W2f [1024, 1024] -> [128, 8, 1024]
    w2f = np.asarray(W2, f32).transpose(1, 0, 2).reshape(1024, 1024)
    w2f = w2f * sg1r[:, None]
    w2m = np.ascontiguousarray(
        w2f.reshape(8, 128, 1024).transpose(1, 0, 2))
    wq1f = np.asarray(Wq1, f32) * sg2r[:, None]
    wq1m = np.ascontiguousarray(
        wq1f.reshape(8, 128, 256).transpose(1, 0, 2))
    wq2m = np.ascontiguousarray(
        np.asarray(Wq2, f32).reshape(2, 128, 128).transpose(1, 0, 2))
    wq3m = np.ascontiguousarray(
        np.tile(np.asarray(Wq3, f32).reshape(1, 1, 128), (128, 4, 1)))

    def pj(v, j):  # [j*128] vector -> [128, j]
        return np.ascontiguousarray(np.asarray(v, f32).reshape(j, 128).T)

    b1m = pj(np.asarray(b1, f32).reshape(1024), 8)
    b2m = pj(np.asarray(b2, f32).reshape(1024), 8)
    bq1m = np.ascontiguousarray(
        np.tile(np.asarray(bq1, f32)[None, :], (128, 1)))
    l1gm = pj(ln1_g, 2)
    l1bm = pj(ln1_b, 2)
    bq2m = np.ascontiguousarray(
        np.tile(np.asarray(bq2, f32)[None, :], (128, 1)))
    l2gm = np.ascontiguousarray(
        np.tile(np.asarray(ln2_g, f32)[None, None, :], (128, 4, 1)))
    l2bm = np.ascontiguousarray(
        np.tile(np.asarray(ln2_b, f32)[None, None, :], (128, 4, 1)))
    bq3m = np.full((128, 1), np.asarray(bq3, f32).reshape(()), f32)

    shared = dict(w1=w1f, b1=b1m, b2=b2m,
                  wq1=wq1m, bq1=bq1m, l1g=l1gm, l1b=l1bm,
                  wq2=wq2m, bq2=bq2m, l2g=l2gm, l2b=l2bm,
                  wq3=wq3m, bq3=bq3m)
    if USE_FP16:
        for k in ("w1", "wq1", "wq2"):
            shared[k] = shared[k].astype(np.float16)
        w2m = w2m.astype(np.float16)
        sa_cores = [sc.astype(np.float16) for sc in sa_cores]
    for c in range(4):
        shared[f"w2c{c}"] = np.ascontiguousarray(w2m[:, :, c * 256:(c + 1) * 256])
    return shared, sa_cores


def make_in_maps(**inputs):
    shared, sa_cores = marshal_inputs(**inputs)
    return [dict(shared, sa=sa_cores[c]) for c in range(N_CORES)]


def assemble_output(results):
    # on-chip y is [128 batch-sub, 128 tile-idx]; host transposes
    return np.concatenate(
        [np.ascontiguousarray(results[c]["y"].T).reshape(B_CORE, 1)
         for c in range(N_CORES)], axis=0)


_NC_CACHE = []


def kernel(**inputs):
    from concourse.bass_utils import run_bass_kernel_spmd

    if not _NC_CACHE:
        _NC_CACHE.append(build_nc())
    nc = _NC_CACHE[0]
    in_maps = make_in_maps(**inputs)
    res = run_bass_kernel_spmd(nc, in_maps, core_ids=list(range(N_CORES)),
                               trace=False)
    return assemble_output(res.results)


# revision 18
# speedup vs baseline: 1.3396x; 1.3396x over previous
"""Self-contained TRN2 Bass kernel for the COR Critic network.

kernel(**inputs) takes the FULL (unsharded) numpy inputs keyed as in
setup_inputs() and returns the FULL [131072, 1] float32 output.

Sharding: pure data parallel over 8 NeuronCores - the batch dim of
state/action is split into 8 equal shards; the (tiny) weights are
replicated. No collectives are needed; per-core outputs are
concatenated on the host.

Implementation notes (per 512-row super-tile, per core):
  - the whole network runs fused on-chip; no intermediate HBM traffic
  - matmul operands in fp16 (PSUM accumulation is fp32); LayerNorm
    statistics and normalization are computed in fp32
  - LayerNorm rstd via DVE Newton iterations (bit-trick seed), keeping
    the ACT engine inside a single activation-table set (tanh/relu)
  - sigmoid gates are folded into the next layer's weight rows on the
    host (pure input marshalling), so no on-chip preamble math
  - LN1 transposes ride the DMA XBAR (dma_start_transpose), not the PE;
    the freed PSUM banks deepen the r2 accumulator rotation (psA=5)
  - three-stage software pipeline: A(p) [r1 riders + r2 + q1 + LN1
    stats] -> TR(p) [XBAR transpose + ACT relu] -> Q2/Btail(p-1); the
    q2 matmuls of pair p-1 issue after pair p's heavy matmuls so the
    PE never waits on the LN1 chain
  - final [128,128] output stays untransposed on-chip; the host
    transposes during unmarshalling
"""

import os

os.environ.setdefault("BASS_NEVER_TRACE", "1")

import numpy as np

import concourse.bacc as bacc
import concourse.bass as bass
import concourse.tile as tile
from concourse import mybir
from concourse.masks import make_identity

F32 = mybir.dt.float32
F32R = mybir.dt.float32r
F16 = mybir.dt.float16
I32 = mybir.dt.int32

# matmul-operand dtype: fp16 halves weight-load time (and enables FWL)
# at ~2e-4 relative rounding; all LayerNorm math stays fp32.
USE_FP16 = True
MMDT = F16 if USE_FP16 else F32R
MMNP = "float16" if USE_FP16 else "float32"
RSQRT_MAGIC = 0x5F3759DF

N_CORES = 8
B_CORE = 16384  # batch rows per core
T = 512         # super-tile batch rows
N_TILES = B_CORE // T
EPS = 1e-5


def build_nc():
    nc = bacc.Bacc("TRN2", target_bir_lowering=False, debug=False,
                   num_devices=N_CORES)

    # DRAM I/O (shapes match host-side pre-marshalled arrays)
    sa = nc.dram_tensor("sa", [N_TILES // 2, 64, T], MMDT, kind="ExternalInput").ap()
    w1 = nc.dram_tensor("w1", [64, 1024], MMDT, kind="ExternalInput").ap()
    b1 = nc.dram_tensor("b1", [128, 8], F32, kind="ExternalInput").ap()
    # w2 pre-chunked on host along the j (output-feature) axis so the
    # first chunk unblocks ripple-2 j=0 early
    w2c = [nc.dram_tensor(f"w2c{c}", [128, 8, 256], MMDT,
                          kind="ExternalInput").ap() for c in range(4)]
    b2 = nc.dram_tensor("b2", [128, 8], F32, kind="ExternalInput").ap()
    wq1 = nc.dram_tensor("wq1", [128, 8, 256], MMDT, kind="ExternalInput").ap()
    bq1 = nc.dram_tensor("bq1", [128, 256], F32, kind="ExternalInput").ap()
    l1g = nc.dram_tensor("l1g", [128, 2], F32, kind="ExternalInput").ap()
    l1b = nc.dram_tensor("l1b", [128, 2], F32, kind="ExternalInput").ap()
    wq2 = nc.dram_tensor("wq2", [128, 2, 128], MMDT, kind="ExternalInput").ap()
    bq2 = nc.dram_tensor("bq2", [128, 128], F32, kind="ExternalInput").ap()
    l2g = nc.dram_tensor("l2g", [128, 4, 128], F32, kind="ExternalInput").ap()
    l2b = nc.dram_tensor("l2b", [128, 4, 128], F32, kind="ExternalInput").ap()
    wq3 = nc.dram_tensor("wq3", [128, 4, 128], F32, kind="ExternalInput").ap()
    bq3 = nc.dram_tensor("bq3", [128, 1], F32, kind="ExternalInput").ap()
    y = nc.dram_tensor("y", [128, 128], F32, kind="ExternalOutput").ap()

    AF = mybir.ActivationFunctionType
    OP = mybir.AluOpType

    with tile.TileContext(nc) as tc:
        with (
            tc.tile_pool(name="consts", bufs=1) as consts,
            tc.tile_pool(name="acts", bufs=2) as acts,
            tc.tile_pool(name="work", bufs=3) as work,
            tc.tile_pool(name="psA", bufs=4, space="PSUM") as psA,
            tc.tile_pool(name="psB", bufs=2, space="PSUM") as psB,
            tc.tile_pool(name="psC", bufs=2, space="PSUM") as psC,
        ):
            # ---------------- preamble: weights to SBUF ----------------
            # All on the sync HWDGE queue; issue order IS the priority
            # order (first slab + r1 weights first so the PE starts
            # within ~3us, then w2 chunk 0 which gates ripple-2 j=0).
            def load(name, shape, dt, src):
                t_ = consts.tile(shape, dt, tag=name)
                nc.sync.dma_start(out=t_, in_=src)
                return t_

            sa2_0 = work.tile([64, T], MMDT, tag="sa_fm")
            nc.sync.dma_start(out=sa2_0, in_=sa[0])
            w1_sb = load("w1", [64, 1024], MMDT, w1)
            b1_sb = load("b1", [128, 8], F32, b1)
            b2_sb = load("b2", [128, 8], F32, b2)
            w2_sb = consts.tile([128, 8, 1024], MMDT, tag="w2")
            for c in range(4):
                nc.sync.dma_start(out=w2_sb[:, :, c * 256:(c + 1) * 256],
                                  in_=w2c[c])
            wq1_sb = load("wq1", [128, 8, 256], MMDT, wq1)
            bq1_sb = load("bq1", [128, 256], F32, bq1)
            l1g_sb = load("l1g", [128, 2], F32, l1g)
            l1b_sb = load("l1b", [128, 2], F32, l1b)
            wq2_sb = load("wq2", [128, 2, 128], MMDT, wq2)
            bq2_sb = load("bq2", [128, 128], F32, bq2)
            l2g_sb = load("l2g", [128, 4, 128], F32, l2g)
            l2b_sb = load("l2b", [128, 4, 128], F32, l2b)
            wq3_sb = load("wq3", [128, 4, 128], F32, wq3)
            bq3_sb = load("bq3", [128, 1], F32, bq3)

            y_all = consts.tile([128, 128], F32, tag="y_all")
            nc.vector.memset(y_all, 0.0)
            magic = consts.tile([128, 4], I32)
            nc.vector.memset(magic, RSQRT_MAGIC)
            # fp16 identity for the last-pair PE-transpose fast path
            ident = consts.tile([128, 128], F32)
            make_identity(nc, ident)
            ident16 = consts.tile([128, 128], MMDT)
            nc.vector.tensor_copy(ident16, ident)

            # Newton rsqrt on DVE (avoids ACT Sqrt: bad ULP + a table-set
            # swap against Tanh every tile). vars_ap: [128, n] variances.
            def rsqrt_dve(vars_ap, n):
                v = work.tile([128, 4], F32, tag="rsq_v")
                nc.vector.tensor_scalar_add(v[:, :n], in0=vars_ap, scalar1=EPS)
                ti = work.tile([128, 4], I32, tag="rsq_t")
                nc.vector.tensor_scalar(
                    ti[:, :n], in0=v[:, :n].bitcast(I32), scalar1=1,
                    scalar2=None, op0=OP.logical_shift_right)
                yn = work.tile([128, 4], F32, tag="rsq_y")
                nc.vector.tensor_sub(yn[:, :n].bitcast(I32), in0=magic[:, :n],
                                     in1=ti[:, :n])
                for _ in range(3):
                    a = work.tile([128, 4], F32, tag="rsq_a")
                    nc.vector.tensor_mul(a[:, :n], in0=yn[:, :n], in1=yn[:, :n])
                    nc.vector.scalar_tensor_tensor(
                        a[:, :n], in0=a[:, :n], scalar=-0.5, in1=v[:, :n],
                        op0=OP.mult, op1=OP.mult)
                    nc.vector.scalar_tensor_tensor(
                        yn[:, :n], in0=a[:, :n], scalar=1.5, in1=yn[:, :n],
                        op0=OP.add, op1=OP.mult)
                return yn

            # ------------- stage A: matmul-heavy front half -------------
            # Pair-structured. r1 matmuls (K=32, single-shot PSUM whose
            # slot frees only at tanh pace) are interleaved one-per-r2-
            # j-group so their PSUM slot is always free when they issue:
            # tile b's r1 rides tile a's r2; the NEXT pair's tile-a r1
            # rides tile b's r2.
            def r1_chunk(x1, sa2, m, j):
                ps = psA.tile([128, T], F32, tag="mm512")
                nc.tensor.matmul(
                    ps, w1_sb[32 * m:32 * (m + 1), j * 128:(j + 1) * 128],
                    sa2[32 * m:32 * (m + 1), :], start=True, stop=True,
                    tile_position=(32 * m, 0))
                nc.scalar.activation(x1[:, j, :], ps, AF.Tanh,
                                     bias=b1_sb[:, j:j + 1])

            def r2_q1(x1, riders):
                # ripple 2: x2 = tanh(W2f'.T @ x1 + b2)  [1024f, Tb]
                x2 = acts.tile([128, 8, T], MMDT, tag="x2")
                for j in range(8):
                    ps = psA.tile([128, T], F32, tag="mm512")
                    for k in range(8):
                        nc.tensor.matmul(
                            ps, w2_sb[:, k, j * 128:(j + 1) * 128],
                            x1[:, k, :], start=(k == 0), stop=(k == 7))
                    nc.scalar.activation(x2[:, j, :], ps, AF.Tanh,
                                         bias=b2_sb[:, j:j + 1])
                    for r in riders:
                        r1_chunk(*r, j)

                # q1 batch-major: z1 = x2.T @ Wq1' + bq1, then LN1 + norm
                z1sb = work.tile([128, 4, 256], F32, tag="z1sb", bufs=4)
                mv1 = work.tile([128, 4, 2], F32, tag="mv1", bufs=2)
                for cp in range(2):
                    zps2 = psB.tile([128, 2, 256], F32, tag="q1")
                    for ci in range(2):
                        c = 2 * cp + ci
                        for k in range(8):
                            nc.tensor.matmul(
                                zps2[:, ci, :], x2[:, k, c * 128:(c + 1) * 128],
                                wq1_sb[:, k, :], start=(k == 0), stop=(k == 7))
                        nc.vector.tensor_add(z1sb[:, c, :], in0=zps2[:, ci, :],
                                             in1=bq1_sb)
                        st = work.tile([128, 6], F32, tag="st1")
                        nc.vector.bn_stats(st, z1sb[:, c, :])
                        nc.vector.bn_aggr(mv1[:, c, :], st)
                rstd1 = rsqrt_dve(mv1[:, :, 1], 4)
                xn1 = work.tile([128, 4, 256], MMDT, tag="xn1", bufs=4)
                for c in range(4):
                    nc.vector.tensor_scalar(
                        xn1[:, c, :], in0=z1sb[:, c, :],
                        scalar1=mv1[:, c, 0:1], scalar2=rstd1[:, c:c + 1],
                        op0=OP.subtract, op1=OP.mult)
                return xn1

            def stage_A_pair(p, x1_a, sa2):
                # resources for the NEXT pair (its tile-a r1 rides r2_b).
                # sa slabs ride the gpsimd SWDGE queue: the sync queue
                # carries the 16 XBAR transposes per pair, which wait on
                # the LN1 normalize - a slab queued behind them would
                # arrive after the riders need it.
                nxt = None
                if p + 1 < N_TILES // 2:
                    sa2n = work.tile([64, T], MMDT, tag="sa_fm")
                    nc.gpsimd.dma_start(out=sa2n, in_=sa[p + 1])
                    x1an = acts.tile([128, 8, T], MMDT, tag="x1", bufs=3)
                    nxt = (x1an, sa2n)

                x1_b = acts.tile([128, 8, T], MMDT, tag="x1", bufs=3)
                riders = [(x1_b, sa2, 1)]
                if nxt:
                    riders.append((nxt[0], nxt[1], 0))
                xn_a = r2_q1(x1_a, riders)
                xn_b = r2_q1(x1_b, [])
                return nxt, [xn_a, xn_b]

            # ------------- stage TP: PE transpose + ACT relu -------------
            # xn1 [128b, 4c, 256f] -> h1T [128f, 2jf, 512b]. The four
            # [128,128] transposes of one jf pack into a single PSUM
            # bank as one accumulation group (start zeroes the bank,
            # the disjoint-offset writes then land on zeroed bytes), so
            # one [128,512] ACT relu drains the whole bank - 2 ACT ops
            # and 2 PSUM banks per tile instead of 8.
            def stage_TP(t, xn1):
                h1T = work.tile([128, 2, T], MMDT, tag="h1T", bufs=4)
                for jf in range(2):
                    X = psC.tile([128, 4, 128], MMDT, tag="tr")
                    for c in range(4):
                        nc.tensor.matmul(
                            X[:, c, :], xn1[:, c, jf * 128:(jf + 1) * 128],
                            ident16, is_transpose=True,
                            start=(c == 0), stop=(c == 3),
                            skip_group_check=True)
                    nc.scalar.activation(
                        h1T[:, jf, :], X, AF.Relu,
                        bias=l1b_sb[:, jf:jf + 1], scale=l1g_sb[:, jf:jf + 1])
                return h1T

            # ------------- stage Q2: q2 matmuls + LN2 normalize -------------
            def stage_Q2(t, h1T):
                z2T = work.tile([128, 4, 128], F32, tag="z2T", bufs=4)
                mv2 = work.tile([128, 4, 2], F32, tag="mv2", bufs=2)
                for cp in range(2):
                    zps2 = psB.tile([128, 2, 128], F32, tag="q1")
                    for ci in range(2):
                        c = 2 * cp + ci
                        for k in range(2):
                            nc.tensor.matmul(
                                zps2[:, ci, :], h1T[:, k, c * 128:(c + 1) * 128],
                                wq2_sb[:, k, :], start=(k == 0), stop=(k == 1))
                        nc.vector.tensor_add(z2T[:, c, :], in0=zps2[:, ci, :],
                                             in1=bq2_sb)
                        st2 = work.tile([128, 6], F32, tag="st2")
                        nc.vector.bn_stats(st2, z2T[:, c, :])
                        nc.vector.bn_aggr(mv2[:, c, :], st2)
                rstd2 = rsqrt_dve(mv2[:, :, 1], 4)
                xn2 = work.tile([128, 4, 128], F32, tag="xn2", bufs=4)
                for c in range(4):
                    nc.vector.tensor_scalar(
                        xn2[:, c, :], in0=z2T[:, c, :], scalar1=mv2[:, c, 0:1],
                        scalar2=rstd2[:, c:c + 1],
                        op0=OP.subtract, op1=OP.mult)
                return xn2

            # ------------- stage B tail: q3 on DVE -------------
            # h2 = relu(xn2 * ln2_g + ln2_b); y = h2 . wq3 (+ bq3 added
            # once at the end). Batch-major on the DVE; the elementwise
            # ops run over all 4 c-quarters at once against host-tiled
            # [128, 4, 128] weight replicas. Results collect into
            # y_all[:, t*4+c].
            def stage_Btail(t, xn2):
                h = work.tile([128, 4, 128], F32, tag="hb")
                nc.vector.tensor_mul(h, in0=xn2, in1=l2g_sb)
                nc.vector.tensor_add(h, in0=h, in1=l2b_sb)
                nc.vector.tensor_scalar_max(h, in0=h, scalar1=0.0)
                nc.vector.tensor_mul(h, in0=h, in1=wq3_sb)
                for c in range(4):
                    idx = t * 4 + c
                    nc.vector.reduce_sum(y_all[:, idx:idx + 1], h[:, c, :],
                                         axis=mybir.AxisListType.X)

            # ---------------- software-pipelined batch loop ----------------
            # Per iteration: heavy A(p), transpose+relu for tile a, then
            # the PREVIOUS pair's q2/LN2/Btail (its h1T has been ready
            # for a whole iteration, and its q2 matmuls give the ACT
            # engine time to drain tile a's relus before tile b's
            # transposes need the PSUM slots back), then tile b.
            NP = N_TILES // 2
            # prologue: pair 0's tile-a r1 runs standalone
            x1a_0 = acts.tile([128, 8, T], MMDT, tag="x1", bufs=3)
            for j in range(8):
                r1_chunk(x1a_0, sa2_0, 0, j)
            pend_a = (x1a_0, sa2_0)
            h1q = {}

            def do_q2bt(p):
                for s in range(2):
                    t_ = 2 * p + s
                    xn2 = stage_Q2(t_, h1q[p][s])
                    stage_Btail(t_, xn2)
                del h1q[p]

            for p in range(NP):
                pend_a, xn1_pair = stage_A_pair(p, *pend_a)
                h_a = stage_TP(2 * p, xn1_pair[0])
                if p >= 1:
                    do_q2bt(p - 1)
                h_b = stage_TP(2 * p + 1, xn1_pair[1])
                h1q[p] = (h_a, h_b)
            do_q2bt(NP - 1)
            nc.vector.tensor_scalar_add(y_all, in0=y_all, scalar1=bq3_sb)
            nc.sync.dma_start(out=y, in_=y_all)

    nc.compile()
    return nc


def marshal_inputs(state, action, W1, b1, g1, W2, b2, g2,
                   Wq1, bq1, ln1_g, ln1_b, Wq2, bq2, ln2_g, ln2_b, Wq3, bq3):
    """Host-side layout marshalling (reshape/transpose/tile + gate folds).

    Returns (shared weight map, per-core list of sa slabs)."""
    f32 = np.float32
    B = state.shape[0]
    assert B == N_CORES * B_CORE

    sa = np.concatenate([np.asarray(state, f32), np.asarray(action, f32)],
                        axis=1)  # [B, 32]
    # per-core: [N_TILES//2, 64, T] feature-major pair slabs
    sa_cores = []
    for cid in range(N_CORES):
        s = sa[cid * B_CORE:(cid + 1) * B_CORE]
        sa_cores.append(np.ascontiguousarray(
            s.reshape(N_TILES // 2, 2, T, 32).transpose(0, 1, 3, 2)
            .reshape(N_TILES // 2, 64, T)))

    # sigmoid gates folded into the NEXT layer's weight rows:
    # (tanh(z)*s) @ W == tanh(z) @ (diag(s) W)
    sg1 = 1.0 / (1.0 + np.exp(-np.asarray(g1, np.float64)))
    sg2 = 1.0 / (1.0 + np.exp(-np.asarray(g2, np.float64)))
    sg1r = np.repeat(sg1.astype(f32), 32)  # [1024] per ripple-1 feature
    sg2r = np.repeat(sg2.astype(f32), 32)

    # W1 [H=32, D=32, K=32] -> W1f [D=32, H*K=1024]
    w1f = np.asarray(W1, f32).transpose(1, 0, 2).reshape(32, 1024)
    w1f = np.ascontiguousarray(np.concatenate([w1f, w1f], axis=0))
    # W2 [H=32, D=1024, K=32] -> diag(sg1) @ W2f [1024, 1024] -> [128, 8, 1024]
    w2f = np.asarray(W2, f32).transpose(1, 0, 2).reshape(1024, 1024)
    w2f = w2f * sg1r[:, None]
    w2m = np.ascontiguousarray(
        w2f.reshape(8, 128, 1024).transpose(1, 0, 2))
    wq1f = np.asarray(Wq1, f32) * sg2r[:, None]
    wq1m = np.ascontiguousarray(
        wq1f.reshape(8, 128, 256).transpose(1, 0, 2))
    wq2m = np.ascontiguousarray(
        np.asarray(Wq2, f32).reshape(2, 128, 128).transpose(1, 0, 2))
    wq3m = np.ascontiguousarray(
        np.tile(np.asarray(Wq3, f32).reshape(1, 1, 128), (128, 4, 1)))

    def pj(v, j):  # [j*128] vector -> [128, j]
        return np.ascontiguousarray(np.asarray(v, f32).reshape(j, 128).T)

    b1m = pj(np.asarray(b1, f32).reshape(1024), 8)
    b2m = pj(np.asarray(b2, f32).reshape(1024), 8)
    bq1m = np.ascontiguousarray(
        np.tile(np.asarray(bq1, f32)[None, :], (128, 1)))
    l1gm = pj(ln1_g, 2)
    l1bm = pj(ln1_b, 2)
    bq2m = np.ascontiguousarray(
        np.tile(np.asarray(bq2, f32)[None, :], (128, 1)))
    l2gm = np.ascontiguousarray(
        np.tile(np.asarray(ln2_g, f32)[None, None, :], (128, 4, 1)))
    l2bm = np.ascontiguousarray(
        np.tile(np.asarray(ln2_b, f32)[None, None, :], (128, 4, 1)))
    bq3m = np.full((128, 1), np.asarray(bq3, f32).reshape(()), f32)

    shared = dict(w1=w1f, b1=b1m, b2=b2m,
                  wq1=wq1m, bq1=bq1m, l1g=l1gm, l1b=l1bm,
                  wq2=wq2m, bq2=bq2m, l2g=l2gm, l2b=l2bm,
                  wq3=wq3m, bq3=bq3m)
    if USE_FP16:
        for k in ("w1", "wq1", "wq2"):
            shared[k] = shared[k].astype(np.float16)
        w2m = w2m.astype(np.float16)
        sa_cores = [sc.astype(np.float16) for sc in sa_cores]
    for c in range(4):
        shared[f"w2c{c}"] = np.ascontiguousarray(w2m[:, :, c * 256:(c + 1) * 256])
    return shared, sa_cores


def make_in_maps(**inputs):
    shared, sa_cores = marshal_inputs(**inputs)
    return [dict(shared, sa=sa_cores[c]) for c in range(N_CORES)]


def assemble_output(results):
    # on-chip y is [128 batch-sub, 128 tile-idx]; host transposes
    return np.concatenate(
        [np.ascontiguousarray(results[c]["y"].T).reshape(B_CORE, 1)
         for c in range(N_CORES)], axis=0)


_NC_CACHE = []


def kernel(**inputs):
    from concourse.bass_utils import run_bass_kernel_spmd

    if not _NC_CACHE:
        _NC_CACHE.append(build_nc())
    nc = _NC_CACHE[0]
    in_maps = make_in_maps(**inputs)
    res = run_bass_kernel_spmd(nc, in_maps, core_ids=list(range(N_CORES)),
                               trace=False)
    return assemble_output(res.results)


# revision 24
# speedup vs baseline: 1.3734x; 1.0252x over previous
"""Self-contained TRN2 Bass kernel for the COR Critic network.

kernel(**inputs) takes the FULL (unsharded) numpy inputs keyed as in
setup_inputs() and returns the FULL [131072, 1] float32 output.

Sharding: pure data parallel over 8 NeuronCores - the batch dim of
state/action is split into 8 equal shards; the (tiny) weights are
replicated. No collectives are needed; per-core outputs are
concatenated on the host.

Implementation notes (per 512-row super-tile, per core):
  - the whole network runs fused on-chip; no intermediate HBM traffic
  - matmul operands in fp16 (PSUM accumulation is fp32); LayerNorm
    statistics and normalization are computed in fp32
  - LayerNorm rstd via DVE Newton iterations (bit-trick seed), keeping
    the ACT engine inside a single activation-table set (tanh/relu)
  - sigmoid gates are folded into the next layer's weight rows on the
    host (pure input marshalling), so no on-chip preamble math
  - LN1 transposes ride the DMA XBAR (dma_start_transpose), not the PE;
    the freed PSUM banks deepen the r2 accumulator rotation (psA=5)
  - three-stage software pipeline: A(p) [r1 riders + r2 + q1 + LN1
    stats] -> TR(p) [XBAR transpose + ACT relu] -> Q2/Btail(p-1); the
    q2 matmuls of pair p-1 issue after pair p's heavy matmuls so the
    PE never waits on the LN1 chain
  - final [128,128] output stays untransposed on-chip; the host
    transposes during unmarshalling
"""

import os

os.environ.setdefault("BASS_NEVER_TRACE", "1")

import numpy as np

import concourse.bacc as bacc
import concourse.bass as bass
import concourse.tile as tile
from concourse import mybir
from concourse.masks import make_identity

F32 = mybir.dt.float32
F32R = mybir.dt.float32r
F16 = mybir.dt.float16
I32 = mybir.dt.int32

# matmul-operand dtype: fp16 halves weight-load time (and enables FWL)
# at ~2e-4 relative rounding; all LayerNorm math stays fp32.
USE_FP16 = True
MMDT = F16 if USE_FP16 else F32R
MMNP = "float16" if USE_FP16 else "float32"
RSQRT_MAGIC = 0x5F3759DF

N_CORES = 8
B_CORE = 16384  # batch rows per core
T = 512         # super-tile batch rows
N_TILES = B_CORE // T
EPS = 1e-5


def build_nc():
    nc = bacc.Bacc("TRN2", target_bir_lowering=False, debug=False,
                   num_devices=N_CORES)

    # DRAM I/O (shapes match host-side pre-marshalled arrays)
    sa = nc.dram_tensor("sa", [N_TILES // 2, 64, T], MMDT, kind="ExternalInput").ap()
    w1 = nc.dram_tensor("w1", [64, 1024], MMDT, kind="ExternalInput").ap()
    b1 = nc.dram_tensor("b1", [128, 8], F32, kind="ExternalInput").ap()
    # w2 pre-chunked on host along the j (output-feature) axis so the
    # first chunk unblocks ripple-2 j=0 early
    w2c = [nc.dram_tensor(f"w2c{c}", [128, 8, 256], MMDT,
                          kind="ExternalInput").ap() for c in range(4)]
    b2 = nc.dram_tensor("b2", [128, 8], F32, kind="ExternalInput").ap()
    wq1 = nc.dram_tensor("wq1", [128, 8, 256], MMDT, kind="ExternalInput").ap()
    bq1 = nc.dram_tensor("bq1", [128, 256], F32, kind="ExternalInput").ap()
    l1g = nc.dram_tensor("l1g", [128, 2], F32, kind="ExternalInput").ap()
    l1b = nc.dram_tensor("l1b", [128, 2], F32, kind="ExternalInput").ap()
    wq2 = nc.dram_tensor("wq2", [128, 2, 128], MMDT, kind="ExternalInput").ap()
    bq2 = nc.dram_tensor("bq2", [128, 128], F32, kind="ExternalInput").ap()
    l2g = nc.dram_tensor("l2g", [128, 4, 128], F32, kind="ExternalInput").ap()
    l2b = nc.dram_tensor("l2b", [128, 4, 128], F32, kind="ExternalInput").ap()
    wq3 = nc.dram_tensor("wq3", [128, 4, 128], F32, kind="ExternalInput").ap()
    y = nc.dram_tensor("y", [128, 128], F32, kind="ExternalOutput").ap()

    AF = mybir.ActivationFunctionType
    OP = mybir.AluOpType

    with tile.TileContext(nc) as tc:
        with (
            tc.tile_pool(name="consts", bufs=1) as consts,
            tc.tile_pool(name="acts", bufs=2) as acts,
            tc.tile_pool(name="work", bufs=3) as work,
            tc.tile_pool(name="psA", bufs=4, space="PSUM") as psA,
            tc.tile_pool(name="psB", bufs=2, space="PSUM") as psB,
            tc.tile_pool(name="psC", bufs=2, space="PSUM") as psC,
        ):
            # ---------------- preamble: weights to SBUF ----------------
            # All on the sync HWDGE queue; issue order IS the priority
            # order (first slab + r1 weights first so the PE starts
            # within ~3us, then w2 chunk 0 which gates ripple-2 j=0).
            def load(name, shape, dt, src):
                t_ = consts.tile(shape, dt, tag=name)
                nc.sync.dma_start(out=t_, in_=src)
                return t_

            sa2_0 = work.tile([64, T], MMDT, tag="sa_fm")
            nc.sync.dma_start(out=sa2_0, in_=sa[0])
            w1_sb = load("w1", [64, 1024], MMDT, w1)
            b1_sb = load("b1", [128, 8], F32, b1)
            b2_sb = load("b2", [128, 8], F32, b2)
            w2_sb = consts.tile([128, 8, 1024], MMDT, tag="w2")
            for c in range(4):
                nc.sync.dma_start(out=w2_sb[:, :, c * 256:(c + 1) * 256],
                                  in_=w2c[c])
            wq1_sb = load("wq1", [128, 8, 256], MMDT, wq1)
            bq1_sb = load("bq1", [128, 256], F32, bq1)
            l1g_sb = load("l1g", [128, 2], F32, l1g)
            l1b_sb = load("l1b", [128, 2], F32, l1b)
            wq2_sb = load("wq2", [128, 2, 128], MMDT, wq2)
            bq2_sb = load("bq2", [128, 128], F32, bq2)
            l2g_sb = load("l2g", [128, 4, 128], F32, l2g)
            l2b_sb = load("l2b", [128, 4, 128], F32, l2b)
            wq3_sb = load("wq3", [128, 4, 128], F32, wq3)

            y_all = consts.tile([128, 128], F32, tag="y_all")
            nc.vector.memset(y_all, 0.0)
            magic = consts.tile([128, 4], I32)
            nc.vector.memset(magic, RSQRT_MAGIC)
            # fp16 identity for the last-pair PE-transpose fast path
            ident = consts.tile([128, 128], F32)
            make_identity(nc, ident)
            ident16 = consts.tile([128, 128], MMDT)
            nc.vector.tensor_copy(ident16, ident)

            # Newton rsqrt on DVE (avoids ACT Sqrt: bad ULP + a table-set
            # swap against Tanh every tile). vars_ap: [128, n] variances.
            def rsqrt_dve(vars_ap, n):
                v = work.tile([128, 4], F32, tag="rsq_v")
                nc.vector.tensor_scalar_add(v[:, :n], in0=vars_ap, scalar1=EPS)
                ti = work.tile([128, 4], I32, tag="rsq_t")
                nc.vector.tensor_scalar(
                    ti[:, :n], in0=v[:, :n].bitcast(I32), scalar1=1,
                    scalar2=None, op0=OP.logical_shift_right)
                yn = work.tile([128, 4], F32, tag="rsq_y")
                nc.vector.tensor_sub(yn[:, :n].bitcast(I32), in0=magic[:, :n],
                                     in1=ti[:, :n])
                for _ in range(2):
                    a = work.tile([128, 4], F32, tag="rsq_a")
                    nc.vector.tensor_mul(a[:, :n], in0=yn[:, :n], in1=yn[:, :n])
                    nc.vector.scalar_tensor_tensor(
                        a[:, :n], in0=a[:, :n], scalar=-0.5, in1=v[:, :n],
                        op0=OP.mult, op1=OP.mult)
                    nc.vector.scalar_tensor_tensor(
                        yn[:, :n], in0=a[:, :n], scalar=1.5, in1=yn[:, :n],
                        op0=OP.add, op1=OP.mult)
                return yn

            # ------------- stage A: matmul-heavy front half -------------
            # Pair-structured. r1 matmuls (K=32, single-shot PSUM whose
            # slot frees only at tanh pace) are interleaved one-per-r2-
            # j-group so their PSUM slot is always free when they issue:
            # tile b's r1 rides tile a's r2; the NEXT pair's tile-a r1
            # rides tile b's r2.
            def r1_chunk(x1, sa2, m, j):
                ps = psA.tile([128, T], F32, tag="mm512")
                nc.tensor.matmul(
                    ps, w1_sb[32 * m:32 * (m + 1), j * 128:(j + 1) * 128],
                    sa2[32 * m:32 * (m + 1), :], start=True, stop=True,
                    tile_position=(32 * m, 0))
                nc.scalar.activation(x1[:, j, :], ps, AF.Tanh,
                                     bias=b1_sb[:, j:j + 1])

            def r2_q1(x1, riders):
                # ripple 2: x2 = tanh(W2f'.T @ x1 + b2)  [1024f, Tb]
                x2 = acts.tile([128, 8, T], MMDT, tag="x2")
                for j in range(8):
                    ps = psA.tile([128, T], F32, tag="mm512")
                    for k in range(8):
                        nc.tensor.matmul(
                            ps, w2_sb[:, k, j * 128:(j + 1) * 128],
                            x1[:, k, :], start=(k == 0), stop=(k == 7))
                    nc.scalar.activation(x2[:, j, :], ps, AF.Tanh,
                                         bias=b2_sb[:, j:j + 1])
                    for r in riders:
                        r1_chunk(*r, j)

                # q1 batch-major: z1 = x2.T @ Wq1' + bq1, then LN1 + norm
                z1sb = work.tile([128, 4, 256], F32, tag="z1sb", bufs=4)
                mv1 = work.tile([128, 4, 2], F32, tag="mv1", bufs=2)
                for cp in range(2):
                    zps2 = psB.tile([128, 2, 256], F32, tag="q1")
                    for ci in range(2):
                        c = 2 * cp + ci
                        for k in range(8):
                            nc.tensor.matmul(
                                zps2[:, ci, :], x2[:, k, c * 128:(c + 1) * 128],
                                wq1_sb[:, k, :], start=(k == 0), stop=(k == 7))
                        nc.vector.tensor_add(z1sb[:, c, :], in0=zps2[:, ci, :],
                                             in1=bq1_sb)
                        st = work.tile([128, 6], F32, tag="st1")
                        nc.vector.bn_stats(st, z1sb[:, c, :])
                        nc.vector.bn_aggr(mv1[:, c, :], st)
                rstd1 = rsqrt_dve(mv1[:, :, 1], 4)
                xn1 = work.tile([128, 4, 256], MMDT, tag="xn1", bufs=4)
                for c in range(4):
                    nc.vector.tensor_scalar(
                        xn1[:, c, :], in0=z1sb[:, c, :],
                        scalar1=mv1[:, c, 0:1], scalar2=rstd1[:, c:c + 1],
                        op0=OP.subtract, op1=OP.mult)
                return xn1

            def stage_A_pair(p, x1_a, sa2):
                # resources for the NEXT pair (its tile-a r1 rides r2_b).
                # sa slabs ride the gpsimd SWDGE queue: the sync queue
                # carries the 16 XBAR transposes per pair, which wait on
                # the LN1 normalize - a slab queued behind them would
                # arrive after the riders need it.
                nxt = None
                if p + 1 < N_TILES // 2:
                    sa2n = work.tile([64, T], MMDT, tag="sa_fm")
                    nc.gpsimd.dma_start(out=sa2n, in_=sa[p + 1])
                    x1an = acts.tile([128, 8, T], MMDT, tag="x1", bufs=3)
                    nxt = (x1an, sa2n)

                x1_b = acts.tile([128, 8, T], MMDT, tag="x1", bufs=3)
                riders = [(x1_b, sa2, 1)]
                if nxt:
                    riders.append((nxt[0], nxt[1], 0))
                xn_a = r2_q1(x1_a, riders)
                xn_b = r2_q1(x1_b, [])
                return nxt, [xn_a, xn_b]

            # ------------- stage TP: PE transpose + ACT relu -------------
            # xn1 [128b, 4c, 256f] -> h1T [128f, 2jf, 512b]. The four
            # [128,128] transposes of one jf pack into a single PSUM
            # bank as one accumulation group (start zeroes the bank,
            # the disjoint-offset writes then land on zeroed bytes), so
            # one [128,512] ACT relu drains the whole bank - 2 ACT ops
            # and 2 PSUM banks per tile instead of 8.
            def stage_TP(t, xn1):
                h1T = work.tile([128, 2, T], MMDT, tag="h1T", bufs=4)
                for jf in range(2):
                    X = psC.tile([128, 4, 128], MMDT, tag="tr")
                    for c in range(4):
                        nc.tensor.matmul(
                            X[:, c, :], xn1[:, c, jf * 128:(jf + 1) * 128],
                            ident16, is_transpose=True,
                            start=(c == 0), stop=(c == 3),
                            skip_group_check=True)
                    nc.scalar.activation(
                        h1T[:, jf, :], X, AF.Relu,
                        bias=l1b_sb[:, jf:jf + 1], scale=l1g_sb[:, jf:jf + 1])
                return h1T

            # ------------- stage Q2: q2 matmuls + LN2 normalize -------------
            def stage_Q2(t, h1T):
                z2T = work.tile([128, 4, 128], F32, tag="z2T", bufs=4)
                mv2 = work.tile([128, 4, 2], F32, tag="mv2", bufs=2)
                for cp in range(2):
                    zps2 = psB.tile([128, 2, 128], F32, tag="q1")
                    for ci in range(2):
                        c = 2 * cp + ci
                        for k in range(2):
                            nc.tensor.matmul(
                                zps2[:, ci, :], h1T[:, k, c * 128:(c + 1) * 128],
                                wq2_sb[:, k, :], start=(k == 0), stop=(k == 1))
                        nc.vector.tensor_add(z2T[:, c, :], in0=zps2[:, ci, :],
                                             in1=bq2_sb)
                        st2 = work.tile([128, 6], F32, tag="st2")
                        nc.vector.bn_stats(st2, z2T[:, c, :])
                        nc.vector.bn_aggr(mv2[:, c, :], st2)
                rstd2 = rsqrt_dve(mv2[:, :, 1], 4)
                xn2 = work.tile([128, 4, 128], F32, tag="xn2", bufs=4)
                for c in range(4):
                    nc.vector.tensor_scalar(
                        xn2[:, c, :], in0=z2T[:, c, :], scalar1=mv2[:, c, 0:1],
                        scalar2=rstd2[:, c:c + 1],
                        op0=OP.subtract, op1=OP.mult)
                return xn2

            # ------------- stage B tail: q3 on DVE -------------
            # h2 = relu(xn2 * ln2_g + ln2_b); y = h2 . wq3 (+ bq3 added
            # once at the end). Batch-major on the DVE; the elementwise
            # ops run over all 4 c-quarters at once against host-tiled
            # [128, 4, 128] weight replicas. Results collect into
            # y_all[:, t*4+c].
            def stage_Btail(t, xn2):
                h = work.tile([128, 4, 128], F32, tag="hb")
                nc.vector.tensor_mul(h, in0=xn2, in1=l2g_sb)
                nc.vector.tensor_add(h, in0=h, in1=l2b_sb)
                nc.vector.tensor_scalar_max(h, in0=h, scalar1=0.0)
                nc.vector.tensor_mul(h, in0=h, in1=wq3_sb)
                for c in range(4):
                    idx = t * 4 + c
                    nc.vector.reduce_sum(y_all[:, idx:idx + 1], h[:, c, :],
                                         axis=mybir.AxisListType.X)

            # ---------------- software-pipelined batch loop ----------------
            # Per iteration: heavy A(p), transpose+relu for tile a, then
            # the PREVIOUS pair's q2/LN2/Btail (its h1T has been ready
            # for a whole iteration, and its q2 matmuls give the ACT
            # engine time to drain tile a's relus before tile b's
            # transposes need the PSUM slots back), then tile b.
            NP = N_TILES // 2
            # prologue: pair 0's tile-a r1 runs standalone
            x1a_0 = acts.tile([128, 8, T], MMDT, tag="x1", bufs=3)
            for j in range(8):
                r1_chunk(x1a_0, sa2_0, 0, j)
            pend_a = (x1a_0, sa2_0)
            h1q = {}

            def do_q2bt(p):
                for s in range(2):
                    t_ = 2 * p + s
                    xn2 = stage_Q2(t_, h1q[p][s])
                    stage_Btail(t_, xn2)
                del h1q[p]

            for p in range(NP):
                if p == NP - 1:
                    # last iteration: drain the previous pair BEFORE the
                    # heavy A so its LN2/Btail DVE chain overlaps A's
                    # matmuls instead of stacking up in the epilogue
                    do_q2bt(p - 1)
                pend_a, xn1_pair = stage_A_pair(p, *pend_a)
                h_a = stage_TP(2 * p, xn1_pair[0])
                if p >= 1 and p < NP - 1:
                    do_q2bt(p - 1)
                h_b = stage_TP(2 * p + 1, xn1_pair[1])
                h1q[p] = (h_a, h_b)
            do_q2bt(NP - 1)
            nc.sync.dma_start(out=y, in_=y_all)

    nc.compile()
    return nc


def marshal_inputs(state, action, W1, b1, g1, W2, b2, g2,
                   Wq1, bq1, ln1_g, ln1_b, Wq2, bq2, ln2_g, ln2_b, Wq3, bq3):
    """Host-side layout marshalling (reshape/transpose/tile + gate folds).

    Returns (shared weight map, per-core list of sa slabs)."""
    f32 = np.float32
    B = state.shape[0]
    assert B == N_CORES * B_CORE

    sa = np.concatenate([np.asarray(state, f32), np.asarray(action, f32)],
                        axis=1)  # [B, 32]
    # per-core: [N_TILES//2, 64, T] feature-major pair slabs
    sa_cores = []
    for cid in range(N_CORES):
        s = sa[cid * B_CORE:(cid + 1) * B_CORE]
        sa_cores.append(np.ascontiguousarray(
            s.reshape(N_TILES // 2, 2, T, 32).transpose(0, 1, 3, 2)
            .reshape(N_TILES // 2, 64, T)))

    # sigmoid gates folded into the NEXT layer's weight rows:
    # (tanh(z)*s) @ W == tanh(z) @ (diag(s) W)
    sg1 = 1.0 / (1.0 + np.exp(-np.asarray(g1, np.float64)))
    sg2 = 1.0 / (1.0 + np.exp(-np.asarray(g2, np.float64)))
    sg1r = np.repeat(sg1.astype(f32), 32)  # [1024] per ripple-1 feature
    sg2r = np.repeat(sg2.astype(f32), 32)

    # W1 [H=32, D=32, K=32] -> W1f [D=32, H*K=1024]
    w1f = np.asarray(W1, f32).transpose(1, 0, 2).reshape(32, 1024)
    w1f = np.ascontiguousarray(np.concatenate([w1f, w1f], axis=0))
    # W2 [H=32, D=1024, K=32] -> diag(sg1) @ W2f [1024, 1024] -> [128, 8, 1024]
    w2f = np.asarray(W2, f32).transpose(1, 0, 2).reshape(1024, 1024)
    w2f = w2f * sg1r[:, None]
    w2m = np.ascontiguousarray(
        w2f.reshape(8, 128, 1024).transpose(1, 0, 2))
    wq1f = np.asarray(Wq1, f32) * sg2r[:, None]
    wq1m = np.ascontiguousarray(
        wq1f.reshape(8, 128, 256).transpose(1, 0, 2))
    wq2m = np.ascontiguousarray(
        np.asarray(Wq2, f32).reshape(2, 128, 128).transpose(1, 0, 2))
    wq3m = np.ascontiguousarray(
        np.tile(np.asarray(Wq3, f32).reshape(1, 1, 128), (128, 4, 1)))

    def pj(v, j):  # [j*128] vector -> [128, j]
        return np.ascontiguousarray(np.asarray(v, f32).reshape(j, 128).T)

    b1m = pj(np.asarray(b1, f32).reshape(1024), 8)
    b2m = pj(np.asarray(b2, f32).reshape(1024), 8)
    bq1m = np.ascontiguousarray(
        np.tile(np.asarray(bq1, f32)[None, :], (128, 1)))
    l1gm = pj(ln1_g, 2)
    l1bm = pj(ln1_b, 2)
    bq2m = np.ascontiguousarray(
        np.tile(np.asarray(bq2, f32)[None, :], (128, 1)))
    l2gm = np.ascontiguousarray(
        np.tile(np.asarray(ln2_g, f32)[None, None, :], (128, 4, 1)))
    l2bm = np.ascontiguousarray(
        np.tile(np.asarray(ln2_b, f32)[None, None, :], (128, 4, 1)))
    shared = dict(w1=w1f, b1=b1m, b2=b2m,
                  wq1=wq1m, bq1=bq1m, l1g=l1gm, l1b=l1bm,
                  wq2=wq2m, bq2=bq2m, l2g=l2gm, l2b=l2bm,
                  wq3=wq3m)
    if USE_FP16:
        for k in ("w1", "wq1", "wq2"):
            shared[k] = shared[k].astype(np.float16)
        w2m = w2m.astype(np.float16)
        sa_cores = [sc.astype(np.float16) for sc in sa_cores]
    for c in range(4):
        shared[f"w2c{c}"] = np.ascontiguousarray(w2m[:, :, c * 256:(c + 1) * 256])
    return shared, sa_cores


_BQ3 = [np.float32(0.0)]


def make_in_maps(**inputs):
    shared, sa_cores = marshal_inputs(**inputs)
    _BQ3[0] = np.float32(np.asarray(inputs["bq3"]).reshape(()))
    return [dict(shared, sa=sa_cores[c]) for c in range(N_CORES)]


def assemble_output(results):
    # on-chip y is [128 batch-sub, 128 tile-idx]; host transposes and
    # adds the scalar q3 bias (folded out of the kernel)
    return _BQ3[0] + np.concatenate(
        [np.ascontiguousarray(results[c]["y"].T).reshape(B_CORE, 1)
         for c in range(N_CORES)], axis=0)


_NC_CACHE = []


def kernel(**inputs):
    from concourse.bass_utils import run_bass_kernel_spmd

    if not _NC_CACHE:
        _NC_CACHE.append(build_nc())
    nc = _NC_CACHE[0]
    in_maps = make_in_maps(**inputs)
    res = run_bass_kernel_spmd(nc, in_maps, core_ids=list(range(N_CORES)),
                               trace=False)
    return assemble_output(res.results)
